# revision 23
# baseline (speedup 1.0000x reference)
"""Angular prototypical loss on 8 TRN2 NeuronCores (Bass/Tile, SPMD).

kernel(**inputs): takes FULL inputs (embeddings [65536,256] f32, labels
[65536] i32, num_classes), shards the batch across the 8 cores, runs one
SPMD Bass kernel (AllReduce of per-class prototype sums on-chip), returns
the scalar mean loss.

Per-core algorithm (B_local = 8192 rows, D = 256, C = 1024):
  Phase A: normalize rows (e_hat), one-hot scatter-matmul accumulating the
    CLASS-SUM TRANSPOSE protoT[d, c] in PSUM (lhsT = e_hat row-chunks,
    moving = one-hot), so the AllReduce output lands directly in the layout
    phase B's matmul needs (no post-AR transposes on the critical path).
  AllReduce protoT (bf16, 512KB).
  Post-AR: column norms of S via ones-matmuls in BOTH layouts
    ([1,C] row for broadcast-scaling shatT; [C/8-part, 8] for the gather
    side), scale shatT = S * pinv (pinv broadcast via 1xK matmul).
    In parallel build a [C, 264] DRAM image of RAW S rows + a pinv column
    for the per-row gather (m = <e_hat, S_label> * pinv[label]).
  Phase B: cos tile = eT.T @ shatT in PSUM; exp via ACT (no accumulator
    read); row sum-of-exp via one fused DVE tensor_tensor_reduce; u via a
    fused TTR against the gathered S rows.
  Epilogue (batched [128, nt]): ArcFace margin phi, Z correction, ln, mean.
"""
import numpy as np
from concourse.bass_utils import run_bass_kernel_spmd

import math

import concourse.bass as bass
import concourse.mybir as mybir
import concourse.tile as tile
import concourse.bacc as bacc

P = 128
D = 256
C = 1024
NCORES = 8
MARGIN = 0.2
INV_T = 10.0
COS_M = math.cos(MARGIN)
SIN_M = math.sin(MARGIN)
TH = math.cos(math.pi - MARGIN)

GATW = 264  # gathered row width: 256 dims + pinv at col 256 + pad

f32 = mybir.dt.float32
bf16 = mybir.dt.bfloat16
fp16 = mybir.dt.float16
i32 = mybir.dt.int32

AF = mybir.ActivationFunctionType
OP = mybir.AluOpType


def build(nt: int = 64):
    BL = P * nt
    ng = nt // 8

    nc = bacc.Bacc("TRN2", target_bir_lowering=False, debug=False,
                   num_devices=NCORES)
    emb = nc.declare_dram_parameter("embeddings", [BL, D], f32, isOutput=False)
    lab = nc.declare_dram_parameter("labels", [BL], i32, isOutput=False)
    out = nc.declare_dram_parameter("out", [P, 1], f32, isOutput=True)

    emb_g = emb.ap().rearrange("(p q) d -> p q d", p=P)      # [128, nt, 256]
    lab_pn = lab.ap().rearrange("(p n) -> p n", p=P)         # [128, nt]

    with tile.TileContext(nc) as tc:
        with (
            tc.tile_pool(name="big", bufs=1) as big,
            tc.tile_pool(name="stage", bufs=12) as stage,
            tc.tile_pool(name="ohp", bufs=4) as ohp,
            tc.tile_pool(name="gat", bufs=4) as gat,
            tc.tile_pool(name="scr", bufs=2) as scr,
            tc.tile_pool(name="dram", bufs=1, space="DRAM") as dram,
        ):
            s_localT = dram.tile([2 * P, C], bf16, tag="s_localT")
            s_globalT = dram.tile([2 * P, C], bf16, tag="s_globalT",
                                  addr_space="Shared")
            s_cd_dram = dram.tile([C, GATW], bf16, tag="s_cd_dram")
            s_localT_v = s_localT.rearrange("(c p) n -> p c n", p=P)
            s_globalT_v = s_globalT.rearrange("(c p) n -> p c n", p=P)
            s_cd_v = s_cd_dram.rearrange("(j p) d -> p j d", p=P)

            # ---- persistent SBUF ----
            e_bf = big.tile([P, nt * D], bf16, tag="e_bf")
            eT = big.tile([P, nt, 2, P], bf16, tag="eT")
            shatT = big.tile([P, 2, C], bf16, tag="shatT")
            sgT = big.tile([P, 2, C], bf16, tag="sgT")
            sq_sb = big.tile([P, 2, C], bf16, tag="sq_sb")
            sg_cd = big.tile([P, 8, GATW], bf16, tag="sg_cd")
            lab_i = big.tile([P, nt], i32, tag="lab_i")
            lab_f = big.tile([P, nt], f32, tag="lab_f")
            normsq = big.tile([P, nt], f32, tag="normsq")
            invn = big.tile([P, nt], f32, tag="invn")
            u_all = big.tile([P, nt], f32, tag="u_all")
            pinv_gat = big.tile([P, nt], f32, tag="pinv_gat")
            sumexp = big.tile([P, nt], f32, tag="sumexp")
            iota16 = big.tile([P, C], fp16, tag="iota16")
            onesq = big.tile([P, P], bf16, tag="onesq")
            inv_bc = big.tile([P, C], f32, tag="inv_bc")
            pinv_bc = big.tile([P, C], bf16, tag="pinv_bc")
            pinv_cb = big.tile([P, 8], bf16, tag="pinv_cb")

            nc.sync.dma_start(out=lab_i[:], in_=lab_pn)
            nc.gpsimd.iota(iota16[:], pattern=[[1, C]], base=0,
                           channel_multiplier=0,
                           allow_small_or_imprecise_dtypes=True)
            nc.vector.tensor_copy(lab_f[:], lab_i[:])
            # all-ones tile
            nc.vector.memset(onesq[:], 1.0)
            nc.vector.memset(sg_cd[:], 0.0)

            # ================= Phase A =================
            with tc.tile_pool(name="psA", bufs=1, space="PSUM") as psA:
                # protoT accumulators: [d-chunk c][class-half h] -> [128, 512]
                proto_ps = [psA.tile([P, 512], f32, tag=f"protoT{c}{h}",
                                     name=f"protoT_ps{c}{h}")
                            for c in range(2) for h in range(2)]
                raws = {}

                def stats(g):
                    # per-pair DMAs so squares start as soon as data lands
                    for pp in range(4):
                        n = g * 8 + 2 * pp
                        raw = stage.tile([P, 2, D], f32, tag="raw",
                                         name=f"raw{n}")
                        nc.sync.dma_start(out=raw[:],
                                          in_=emb_g[:, n:n + 2, :])
                        raws[g * 4 + pp] = raw
                        for t in range(2):
                            sqt = scr.tile([P, D], f32, tag="sq")
                            nc.scalar.activation(
                                sqt[:], raw[:, t, :], AF.Square,
                                accum_out=normsq[:, n + t:n + t + 1])
                    tmp8 = scr.tile([P, 8], f32, tag="tmp8")
                    gsl = slice(g * 8, (g + 1) * 8)
                    nc.vector.reciprocal(tmp8[:], normsq[:, gsl])
                    nc.scalar.sqrt(invn[:, gsl], tmp8[:])

                def consume(g):
                    for t in range(8):
                        n = g * 8 + t
                        raw = raws[g * 4 + t // 2]
                        e_n = e_bf[:, n * D:(n + 1) * D]
                        nc.vector.tensor_scalar(
                            e_n, raw[:, t % 2, :], invn[:, n:n + 1], None,
                            OP.mult)
                        oh = ohp.tile([P, C], bf16, tag="oh")
                        nc.vector.tensor_scalar(
                            oh[:], iota16[:], lab_f[:, n:n + 1], None,
                            OP.is_equal)
                        for c in range(2):
                            for h in range(2):
                                nc.tensor.matmul(
                                    out=proto_ps[c * 2 + h][:],
                                    lhsT=e_n[:, c * P:(c + 1) * P],
                                    rhs=oh[:, h * 512:(h + 1) * 512],
                                    start=(n == 0), stop=(n == nt - 1))

                stats(0)
                stats(1)
                for g in range(ng):
                    if g + 2 < ng:
                        stats(g + 2)
                    consume(g)

                # proto epilogue: PSUM -> SBUF bf16 in [d, c] layout
                sT_sb = big.tile([P, 2, C], bf16, tag="sT_sb")
                for c in range(2):
                    for h in range(2):
                        nc.vector.tensor_copy(
                            sT_sb[:, c, h * 512:(h + 1) * 512],
                            proto_ps[c * 2 + h][:])

            # ---- DRAM -> AllReduce -> back ----
            nc.sync.dma_start(out=s_localT_v, in_=sT_sb[:])
            nc.gpsimd.collective_compute(
                "AllReduce", OP.add,
                replica_groups=[list(range(NCORES))],
                ins=[s_localT[:].opt()], outs=[s_globalT[:].opt()])
            # eT transposes: needed only once phase B matmuls start; they
            # do not depend on the AllReduce, so they run inside its window.
            for g in range(ng):
                nc.sync.dma_start_transpose(
                    out=eT[:, g * 8:(g + 1) * 8, :, :],
                    in_=e_bf[:, g * 8 * D:(g + 1) * 8 * D])
            nc.sync.dma_start(out=sgT[:], in_=s_globalT_v)

            # ---- raw-S transposes for the gather image (they gate the
            #      gathers); xbar transpose needs a contiguous output
            #      tile; copy into the strided image afterwards. ----
            sg_tmp = big.tile([P, 2, 8, P], bf16, tag="sg_tmp")
            for c in range(2):
                nc.sync.dma_start_transpose(
                    out=sg_tmp[:, c, :, :],
                    in_=sgT[:, c, :])
            for c in range(2):
                nc.vector.tensor_copy(
                    sg_cd[:, :, c * P:(c + 1) * P], sg_tmp[:, c, :, :])

            pinv_dram = dram.tile([C], bf16, tag="pinv_dram")
            with tc.tile_pool(name="psN", bufs=1, space="PSUM") as psN:
                nsq_bc = psN.tile([P, C], f32, tag="nsq_bc")

                # squares of the reduced sums (bf16)
                nc.vector.tensor_tensor(sq_sb[:], sgT[:], sgT[:], op=OP.mult)
                # column sums, broadcast to every partition: ones^T @ sq
                for h in range(2):
                    for c in range(2):
                        nc.tensor.matmul(
                            out=nsq_bc[:, h * 512:(h + 1) * 512],
                            lhsT=onesq[:],
                            rhs=sq_sb[:, c, h * 512:(h + 1) * 512],
                            start=(c == 0), stop=(c == 1))
                # pinv = sqrt(1/nsq), broadcast across partitions already
                nc.vector.reciprocal(inv_bc[:], nsq_bc[:])
                nc.scalar.sqrt(pinv_bc[:], inv_bc[:])
                # shatT = S * pinv (per-class scale)
                for c in range(2):
                    nc.vector.tensor_tensor(
                        shatT[:, c, :], sgT[:, c, :], pinv_bc[:],
                        op=OP.mult)

                # pinv column for the gather image: roundtrip through DRAM
                # to reshape [1, C] -> [128, 8] (class-partition layout)
                nc.sync.dma_start(out=pinv_dram[:], in_=pinv_bc[0:1, :])
                nc.sync.dma_start(
                    out=pinv_cb[:],
                    in_=pinv_dram.rearrange("(j p) -> p j", p=P))
                nc.vector.tensor_copy(sg_cd[:, :, 256:257],
                                      pinv_cb[:].unsqueeze(2))
                nc.sync.dma_start(out=s_cd_v, in_=sg_cd[:])

            # ================= Phase B =================
            with tc.tile_pool(name="psB", bufs=3, space="PSUM") as psB:
                for g in range(ng):
                    Gg = gat.tile([P, 8, GATW], bf16, tag="Gg")
                    for t in range(8):
                        n = g * 8 + t
                        nc.gpsimd.indirect_dma_start(
                            out=Gg[:, t, :], out_offset=None,
                            in_=s_cd_dram[:],
                            in_offset=bass.IndirectOffsetOnAxis(
                                ap=lab_i[:, n:n + 1], axis=0))
                    for t in range(8):
                        n = g * 8 + t
                        cos_ps = psB.tile([P, C], f32, tag="cos")
                        for h in range(2):
                            for c in range(2):
                                nc.tensor.matmul(
                                    out=cos_ps[:, h * 512:(h + 1) * 512],
                                    lhsT=eT[:, n, c, :],
                                    rhs=shatT[:, c, h * 512:(h + 1) * 512],
                                    start=(c == 0), stop=(c == 1))
                        exps = scr.tile([P, C], bf16, tag="exps")
                        nc.scalar.activation(exps[:], cos_ps[:], AF.Exp,
                                             scale=INV_T,
                                             accum_out=sumexp[:, n:n + 1])
                        mjunk = scr.tile([P, D], bf16, tag="mjunk")
                        nc.vector.tensor_tensor(
                            mjunk[:], e_bf[:, n * D:(n + 1) * D],
                            Gg[:, t, 0:D], op=OP.mult)
                        nc.vector.reduce_sum(u_all[:, n:n + 1], mjunk[:],
                                             axis=mybir.AxisListType.X)
                    # per-group pinv pickup (strided [128, 8] copy)
                    nc.vector.tensor_copy(
                        pinv_gat[:, g * 8:(g + 1) * 8],
                        Gg[:, :, 256:257].squeeze(2))

            # ================= epilogue (batched [P, nt]) ========
            m_all = big.tile([P, nt], f32, tag="m_all")
            b1 = big.tile([P, nt], f32, tag="b1")
            b2 = big.tile([P, nt], f32, tag="b2")
            b3 = big.tile([P, nt], f32, tag="b3")
            b4 = big.tile([P, nt], f32, tag="b4")
            mask = big.tile([P, nt], mybir.dt.uint8, tag="mask")
            phi_f = big.tile([P, nt], f32, tag="phi_f")

            nc.vector.tensor_tensor(m_all[:], u_all[:], pinv_gat[:],
                                    op=OP.mult)
            nc.vector.tensor_tensor(b1[:], m_all[:], m_all[:], op=OP.mult)
            nc.vector.tensor_scalar(b1[:], b1[:], -1.0, 1.0, OP.mult, OP.add)
            nc.vector.tensor_scalar_max(b1[:], b1[:], 1e-12)
            # sin = exp(0.5*ln(1-m^2)): stays in the ln/exp table set
            nc.scalar.activation(b2[:], b1[:], AF.Ln, scale=1.0)
            nc.scalar.activation(b2[:], b2[:], AF.Exp, scale=0.5)
            nc.vector.tensor_scalar_mul(b3[:], m_all[:], COS_M)
            nc.vector.tensor_scalar(b2[:], b2[:], -SIN_M, None, OP.mult)
            nc.vector.tensor_add(b3[:], b3[:], b2[:])           # phi
            nc.vector.tensor_scalar(mask[:], m_all[:], TH, None, OP.is_gt)
            nc.vector.tensor_scalar(b4[:], m_all[:], -MARGIN, None, OP.add)
            nc.vector.select(phi_f[:], mask[:], b3[:], b4[:])
            nc.scalar.activation(b1[:], m_all[:], AF.Exp, scale=INV_T)
            nc.scalar.activation(b2[:], phi_f[:], AF.Exp, scale=INV_T)
            nc.vector.tensor_sub(b1[:], sumexp[:], b1[:])
            nc.vector.tensor_add(b1[:], b1[:], b2[:])           # Z
            nc.scalar.activation(b2[:], b1[:], AF.Ln, scale=1.0)
            nc.vector.tensor_scalar_mul(b3[:], phi_f[:], INV_T)
            nc.vector.tensor_sub(b2[:], b2[:], b3[:])           # nll
            part = big.tile([P, 1], f32, tag="part")
            nc.vector.reduce_sum(part[:], b2[:], axis=mybir.AxisListType.X)
            nc.sync.dma_start(out=out[:], in_=part[:])

    nc.compile()
    return nc


_NC_CACHE = {}


def kernel(embeddings, labels, num_classes=None, **_ignored):
    embeddings = np.ascontiguousarray(embeddings, dtype=np.float32)
    labels = np.ascontiguousarray(labels, dtype=np.int32)
    B = embeddings.shape[0]
    BL = B // NCORES

    if "nc" not in _NC_CACHE:
        _NC_CACHE["nc"] = build()
    nc = _NC_CACHE["nc"]

    in_maps = [{"embeddings": embeddings[i * BL:(i + 1) * BL],
                "labels": labels[i * BL:(i + 1) * BL]}
               for i in range(NCORES)]
    res = run_bass_kernel_spmd(nc, in_maps, list(range(NCORES)))
    total = 0.0
    for i in range(NCORES):
        total += res.results[i]["out"].astype(np.float64).sum()
    return np.float32(total / B)


# revision 25
# speedup vs baseline: 1.0966x; 1.0966x over previous
"""Angular prototypical loss on 8 TRN2 NeuronCores (Bass/Tile, SPMD).

kernel(**inputs): takes FULL inputs (embeddings [65536,256] f32, labels
[65536] i32, num_classes), shards the batch across the 8 cores, runs one
SPMD Bass kernel (AllReduce of per-class prototype sums on-chip), returns
the scalar mean loss.

Per-core algorithm (B_local = 8192 rows, D = 256, C = 1024):
  Phase A: normalize rows (e_hat), one-hot scatter-matmul accumulating the
    CLASS-SUM TRANSPOSE protoT[d, c] in PSUM (lhsT = e_hat row-chunks,
    moving = one-hot), so the AllReduce output lands directly in the layout
    phase B's matmul needs (no post-AR transposes on the critical path).
  AllReduce protoT (bf16, 512KB).
  Post-AR: column norms of S via ones-matmuls in BOTH layouts
    ([1,C] row for broadcast-scaling shatT; [C/8-part, 8] for the gather
    side), scale shatT = S * pinv (pinv broadcast via 1xK matmul).
    In parallel build a [C, 264] DRAM image of RAW S rows + a pinv column
    for the per-row gather (m = <e_hat, S_label> * pinv[label]).
  Phase B: cos tile = eT.T @ shatT in PSUM; exp via ACT (no accumulator
    read); row sum-of-exp via one fused DVE tensor_tensor_reduce; u via a
    fused TTR against the gathered S rows.
  Epilogue (batched [128, nt]): ArcFace margin phi, Z correction, ln, mean.
"""
import numpy as np
from concourse.bass_utils import run_bass_kernel_spmd

import math

import concourse.bass as bass
import concourse.mybir as mybir
import concourse.tile as tile
import concourse.bacc as bacc

P = 128
D = 256
C = 1024
NCORES = 8
MARGIN = 0.2
INV_T = 10.0
COS_M = math.cos(MARGIN)
SIN_M = math.sin(MARGIN)
TH = math.cos(math.pi - MARGIN)

GATW = 264  # gathered row width: 256 dims + pinv at col 256 + pad

f32 = mybir.dt.float32
bf16 = mybir.dt.bfloat16
fp16 = mybir.dt.float16
i32 = mybir.dt.int32

AF = mybir.ActivationFunctionType
OP = mybir.AluOpType


def build(nt: int = 64):
    BL = P * nt
    ng = nt // 8

    nc = bacc.Bacc("TRN2", target_bir_lowering=False, debug=False,
                   num_devices=NCORES)
    emb = nc.declare_dram_parameter("embeddings", [BL, D], f32, isOutput=False)
    lab = nc.declare_dram_parameter("labels", [BL], i32, isOutput=False)
    out = nc.declare_dram_parameter("out", [P, 1], f32, isOutput=True)

    emb_g = emb.ap().rearrange("(p q) d -> p q d", p=P)      # [128, nt, 256]
    lab_pn = lab.ap().rearrange("(p n) -> p n", p=P)         # [128, nt]

    with tile.TileContext(nc) as tc:
        with (
            tc.tile_pool(name="big", bufs=1) as big,
            tc.tile_pool(name="stage", bufs=12) as stage,
            tc.tile_pool(name="ohp", bufs=4) as ohp,
            tc.tile_pool(name="gat", bufs=4) as gat,
            tc.tile_pool(name="scr", bufs=2) as scr,
            tc.tile_pool(name="dram", bufs=1, space="DRAM") as dram,
        ):
            s_localT = dram.tile([2 * P, C], bf16, tag="s_localT")
            s_globalT = dram.tile([2 * P, C], bf16, tag="s_globalT",
                                  addr_space="Shared")
            s_cd_dram = dram.tile([C, GATW], bf16, tag="s_cd_dram")
            s_localT_v = s_localT.rearrange("(c p) n -> p c n", p=P)
            s_globalT_v = s_globalT.rearrange("(c p) n -> p c n", p=P)
            s_cd_v = s_cd_dram.rearrange("(j p) d -> p j d", p=P)

            # ---- persistent SBUF ----
            e_bf = big.tile([P, nt * D], bf16, tag="e_bf")
            eT = big.tile([P, nt, 2, P], bf16, tag="eT")
            shatT = big.tile([P, 2, C], bf16, tag="shatT")
            sgT = big.tile([P, 2, C], bf16, tag="sgT")
            sq_sb = big.tile([P, 2, C], bf16, tag="sq_sb")
            sg_cd = big.tile([P, 8, GATW], bf16, tag="sg_cd")
            lab_i = big.tile([P, nt], i32, tag="lab_i")
            lab_f = big.tile([P, nt], f32, tag="lab_f")
            normsq = big.tile([P, nt], f32, tag="normsq")
            invn = big.tile([P, nt], f32, tag="invn")
            u_all = big.tile([P, nt], f32, tag="u_all")
            pinv_gat = big.tile([P, nt], f32, tag="pinv_gat")
            sumexp = big.tile([P, nt], f32, tag="sumexp")
            iota16 = big.tile([P, C], fp16, tag="iota16")
            onesq = big.tile([P, P], bf16, tag="onesq")
            inv_bc = big.tile([P, C], f32, tag="inv_bc")
            pinv_bc = big.tile([P, C], bf16, tag="pinv_bc")
            pinv_cb = big.tile([P, 8], bf16, tag="pinv_cb")

            nc.sync.dma_start(out=lab_i[:], in_=lab_pn)
            nc.gpsimd.iota(iota16[:], pattern=[[1, C]], base=0,
                           channel_multiplier=0,
                           allow_small_or_imprecise_dtypes=True)
            nc.vector.tensor_copy(lab_f[:], lab_i[:])
            # all-ones tile
            nc.vector.memset(onesq[:], 1.0)
            nc.vector.memset(sg_cd[:], 0.0)

            # ================= Phase A =================
            with tc.tile_pool(name="psA", bufs=1, space="PSUM") as psA:
                # protoT accumulators: [d-chunk c][class-half h] -> [128, 512]
                proto_ps = [psA.tile([P, 512], f32, tag=f"protoT{c}{h}",
                                     name=f"protoT_ps{c}{h}")
                            for c in range(2) for h in range(2)]
                raws = {}

                def stats(g):
                    # per-pair DMAs so squares start as soon as data lands
                    for pp in range(4):
                        n = g * 8 + 2 * pp
                        raw = stage.tile([P, 2, D], f32, tag="raw",
                                         name=f"raw{n}")
                        nc.sync.dma_start(out=raw[:],
                                          in_=emb_g[:, n:n + 2, :])
                        raws[g * 4 + pp] = raw
                        for t in range(2):
                            sqt = scr.tile([P, D], f32, tag="sq")
                            nc.scalar.activation(
                                sqt[:], raw[:, t, :], AF.Square,
                                accum_out=normsq[:, n + t:n + t + 1])
                    tmp8 = scr.tile([P, 8], f32, tag="tmp8")
                    gsl = slice(g * 8, (g + 1) * 8)
                    nc.vector.reciprocal(tmp8[:], normsq[:, gsl])
                    nc.scalar.sqrt(invn[:, gsl], tmp8[:])

                def consume(g):
                    for t in range(8):
                        n = g * 8 + t
                        raw = raws[g * 4 + t // 2]
                        e_n = e_bf[:, n * D:(n + 1) * D]
                        nc.vector.tensor_scalar(
                            e_n, raw[:, t % 2, :], invn[:, n:n + 1], None,
                            OP.mult)
                        oh = ohp.tile([P, C], bf16, tag="oh")
                        nc.vector.tensor_scalar(
                            oh[:], iota16[:], lab_f[:, n:n + 1], None,
                            OP.is_equal)
                        for c in range(2):
                            for h in range(2):
                                nc.tensor.matmul(
                                    out=proto_ps[c * 2 + h][:],
                                    lhsT=e_n[:, c * P:(c + 1) * P],
                                    rhs=oh[:, h * 512:(h + 1) * 512],
                                    start=(n == 0), stop=(n == nt - 1))

                stats(0)
                stats(1)
                for g in range(ng):
                    if g + 2 < ng:
                        stats(g + 2)
                    consume(g)

                # proto epilogue: PSUM -> SBUF bf16 in [d, c] layout
                sT_sb = big.tile([P, 2, C], bf16, tag="sT_sb")
                for c in range(2):
                    for h in range(2):
                        nc.vector.tensor_copy(
                            sT_sb[:, c, h * 512:(h + 1) * 512],
                            proto_ps[c * 2 + h][:])

            # ---- DRAM -> AllReduce -> back ----
            nc.sync.dma_start(out=s_localT_v, in_=sT_sb[:])
            nc.gpsimd.collective_compute(
                "AllReduce", OP.add,
                replica_groups=[list(range(NCORES))],
                ins=[s_localT[:].opt()], outs=[s_globalT[:].opt()])
            # eT transposes: needed only once phase B matmuls start; they
            # do not depend on the AllReduce, so they run inside its window.
            for g in range(ng):
                nc.sync.dma_start_transpose(
                    out=eT[:, g * 8:(g + 1) * 8, :, :],
                    in_=e_bf[:, g * 8 * D:(g + 1) * 8 * D])
            nc.sync.dma_start(out=sgT[:], in_=s_globalT_v)

            # ---- raw-S transposes for the gather image (they gate the
            #      gathers); xbar transpose needs a contiguous output
            #      tile; copy into the strided image afterwards. ----
            sg_tmp = big.tile([P, 2, 8, P], bf16, tag="sg_tmp")
            for c in range(2):
                nc.sync.dma_start_transpose(
                    out=sg_tmp[:, c, :, :],
                    in_=sgT[:, c, :])
            for c in range(2):
                nc.vector.tensor_copy(
                    sg_cd[:, :, c * P:(c + 1) * P], sg_tmp[:, c, :, :])

            pinv_dram = dram.tile([C], bf16, tag="pinv_dram")
            with tc.tile_pool(name="psN", bufs=1, space="PSUM") as psN:
                nsq_bc = psN.tile([P, C], f32, tag="nsq_bc")

                # squares of the reduced sums (bf16)
                nc.vector.tensor_tensor(sq_sb[:], sgT[:], sgT[:], op=OP.mult)
                # column sums, broadcast to every partition: ones^T @ sq
                for h in range(2):
                    for c in range(2):
                        nc.tensor.matmul(
                            out=nsq_bc[:, h * 512:(h + 1) * 512],
                            lhsT=onesq[:],
                            rhs=sq_sb[:, c, h * 512:(h + 1) * 512],
                            start=(c == 0), stop=(c == 1))
                # pinv = sqrt(1/nsq), broadcast across partitions already
                nc.vector.reciprocal(inv_bc[:], nsq_bc[:])
                nc.scalar.sqrt(pinv_bc[:], inv_bc[:])
                # shatT = S * pinv (per-class scale)
                for c in range(2):
                    nc.vector.tensor_tensor(
                        shatT[:, c, :], sgT[:, c, :], pinv_bc[:],
                        op=OP.mult)

                # pinv column for the gather image: roundtrip through DRAM
                # to reshape [1, C] -> [128, 8] (class-partition layout)
                nc.sync.dma_start(out=pinv_dram[:], in_=pinv_bc[0:1, :])
                nc.sync.dma_start(
                    out=pinv_cb[:],
                    in_=pinv_dram.rearrange("(j p) -> p j", p=P))
                nc.vector.tensor_copy(sg_cd[:, :, 256:257],
                                      pinv_cb[:].unsqueeze(2))
                nc.sync.dma_start(out=s_cd_v, in_=sg_cd[:])

            # ================= Phase B =================
            with tc.tile_pool(name="psB", bufs=3, space="PSUM") as psB:
                for g in range(ng):
                    Gg = gat.tile([P, 8, GATW], bf16, tag="Gg")
                    for t in range(8):
                        n = g * 8 + t
                        nc.gpsimd.indirect_dma_start(
                            out=Gg[:, t, :], out_offset=None,
                            in_=s_cd_dram[:],
                            in_offset=bass.IndirectOffsetOnAxis(
                                ap=lab_i[:, n:n + 1], axis=0))
                    for t in range(8):
                        n = g * 8 + t
                        cos_ps = psB.tile([P, C], f32, tag="cos")
                        for h in range(2):
                            for c in range(2):
                                nc.tensor.matmul(
                                    out=cos_ps[:, h * 512:(h + 1) * 512],
                                    lhsT=eT[:, n, c, :],
                                    rhs=shatT[:, c, h * 512:(h + 1) * 512],
                                    start=(c == 0), stop=(c == 1))
                        exps = scr.tile([P, C], bf16, tag="exps")
                        nc.scalar.activation(exps[:], cos_ps[:], AF.Exp,
                                             scale=INV_T,
                                             accum_out=sumexp[:, n:n + 1])
                        mjunk = scr.tile([P, D], bf16, tag="mjunk")
                        nc.vector.tensor_tensor(
                            mjunk[:], e_bf[:, n * D:(n + 1) * D],
                            Gg[:, t, 0:D], op=OP.mult)
                        nc.vector.reduce_sum(u_all[:, n:n + 1], mjunk[:],
                                             axis=mybir.AxisListType.X)
                    # per-group pinv pickup (strided [128, 8] copy)
                    nc.vector.tensor_copy(
                        pinv_gat[:, g * 8:(g + 1) * 8],
                        Gg[:, :, 256:257].squeeze(2))

            # ================= epilogue (batched [P, nt]) ========
            m_all = big.tile([P, nt], f32, tag="m_all")
            b1 = big.tile([P, nt], f32, tag="b1")
            b2 = big.tile([P, nt], f32, tag="b2")
            b3 = big.tile([P, nt], f32, tag="b3")
            b4 = big.tile([P, nt], f32, tag="b4")
            mask = big.tile([P, nt], mybir.dt.uint8, tag="mask")
            phi_f = big.tile([P, nt], f32, tag="phi_f")

            nc.vector.tensor_tensor(m_all[:], u_all[:], pinv_gat[:],
                                    op=OP.mult)
            nc.vector.tensor_tensor(b1[:], m_all[:], m_all[:], op=OP.mult)
            nc.vector.tensor_scalar(b1[:], b1[:], -1.0, 1.0, OP.mult, OP.add)
            nc.vector.tensor_scalar_max(b1[:], b1[:], 1e-12)
            # sin = exp(0.5*ln(1-m^2)): stays in the ln/exp table set
            nc.scalar.activation(b2[:], b1[:], AF.Ln, scale=1.0)
            nc.scalar.activation(b2[:], b2[:], AF.Exp, scale=0.5)
            nc.vector.tensor_scalar_mul(b3[:], m_all[:], COS_M)
            nc.vector.tensor_scalar(b2[:], b2[:], -SIN_M, None, OP.mult)
            nc.vector.tensor_add(b3[:], b3[:], b2[:])           # phi
            nc.vector.tensor_scalar(mask[:], m_all[:], TH, None, OP.is_gt)
            nc.vector.tensor_scalar(b4[:], m_all[:], -MARGIN, None, OP.add)
            nc.vector.select(phi_f[:], mask[:], b3[:], b4[:])
            nc.scalar.activation(b1[:], m_all[:], AF.Exp, scale=INV_T)
            nc.scalar.activation(b2[:], phi_f[:], AF.Exp, scale=INV_T)
            nc.vector.tensor_sub(b1[:], sumexp[:], b1[:])
            nc.vector.tensor_add(b1[:], b1[:], b2[:])           # Z
            nc.scalar.activation(b2[:], b1[:], AF.Ln, scale=1.0)
            nc.vector.tensor_scalar_mul(b3[:], phi_f[:], INV_T)
            nc.vector.tensor_sub(b2[:], b2[:], b3[:])           # nll
            part = big.tile([P, 1], f32, tag="part")
            nc.vector.reduce_sum(part[:], b2[:], axis=mybir.AxisListType.X)
            nc.sync.dma_start(out=out[:], in_=part[:])

    nc.compile()
    return nc


_NC_CACHE = {}


def kernel(embeddings, labels, num_classes=None, **_ignored):
    embeddings = np.ascontiguousarray(embeddings, dtype=np.float32)
    labels = np.ascontiguousarray(labels, dtype=np.int32)
    B = embeddings.shape[0]
    BL = B // NCORES

    if "nc" not in _NC_CACHE:
        _NC_CACHE["nc"] = build()
    nc = _NC_CACHE["nc"]

    in_maps = [{"embeddings": embeddings[i * BL:(i + 1) * BL],
                "labels": labels[i * BL:(i + 1) * BL]}
               for i in range(NCORES)]
    res = run_bass_kernel_spmd(nc, in_maps, list(range(NCORES)))
    total = 0.0
    for i in range(NCORES):
        total += res.results[i]["out"].astype(np.float64).sum()
    return np.float32(total / B)
